# revision 1
# baseline (speedup 1.0000x reference)
"""DGCNN kernel for Trainium2 (Bass/Tile), data-parallel over batch across 8 cores.

Key algorithmic decomposition (per core, one point cloud of N=2048 points):
  EdgeConv(max_k relu(bn(W @ [nb - ctr; ctr]))) decomposes as
    h[n,k,o] = Wn·p_{idx(n,k)} + (Wc - Wn)·p_n         (Wn = W[:, :C], Wc = W[:, C:])
  and since BN scale > 0 and relu is monotonic,
    out[n] = relu(bn(max_k (Wn·p_idx) + Wd·p_n))       (Wd = Wc - Wn)
  So per layer: Y = P @ WnT (one matmul over points), gather+max over the 20
  kNN rows of Y, add the center term, bn+relu.

  kNN: keys[n,m] = 2*(p_n·p_m) - |p_n|^2 - |p_m|^2 (= -d2), built fully on the
  PE via rank-1 correction matmuls; exact top-24 per row via 3 rounds of DVE
  max8 / max_index / match_replace; top-20 = first 20 (sorted desc).

  Gather runs on GPSIMD (ap_gather) against Y^T [Cout, N]; the index list is
  rewrapped to the per-core [16, S] partition-interleaved layout with two
  small SBUF->SBUF DMA steps (partition fold + replicate).
"""

import sys

import numpy as np

sys.path.insert(0, "/opt/trn_rl_repo")

EPS = 1e-5
BN_SCALE = np.float32(1.0 / np.sqrt(1.0 + EPS))
N = 2048
K = 20
KR = 24  # extracted per row (3 rounds of max8)
NCORES = 8
DIMS = [(3, 64), (64, 64), (64, 128), (128, 256)]  # (Cin, Cout) per edge conv

_CACHE = {}


def _build_module(repeat=1):
    import contextlib

    from concourse import bacc, mybir, tile

    dt = mybir.dt
    f32 = dt.float32
    f32r = dt.float32r
    f16 = dt.float16
    u16 = dt.uint16
    i16 = dt.int16
    AF = mybir.ActivationFunctionType
    ALU = mybir.AluOpType
    AX = mybir.AxisListType

    nc = bacc.Bacc("TRN2", target_bir_lowering=False, debug=False)

    # ---------------- DRAM I/O ----------------
    X = nc.dram_tensor("xq", [3, N], f32, kind="ExternalInput")
    conv_w = []
    for li, (ci, co) in enumerate(DIMS):
        nb = (co + 127) // 128
        cb = min(co, 128)
        conv_w.append(
            dict(
                wnT=nc.dram_tensor(
                    f"wn{li}", [ci, 128 if co == 64 else co], f32, kind="ExternalInput"
                ),
                wdT=nc.dram_tensor(f"wd{li}", [ci, co], f32, kind="ExternalInput"),
                gs=nc.dram_tensor(f"gs{li}", [cb, nb], f32, kind="ExternalInput"),
                bb=nc.dram_tensor(f"bb{li}", [cb, nb], f32, kind="ExternalInput"),
            )
        )
    W5 = nc.dram_tensor("w5t", [128, 5, 1024], f32, kind="ExternalInput")
    G5 = nc.dram_tensor("g5s", [128, 8], f32, kind="ExternalInput")
    B5 = nc.dram_tensor("b5s", [128, 8], f32, kind="ExternalInput")
    WL1 = nc.dram_tensor("wl1t", [128, 16, 512], f32, kind="ExternalInput")
    G6 = nc.dram_tensor("g6s", [128, 4], f32, kind="ExternalInput")
    B6 = nc.dram_tensor("b6s", [128, 4], f32, kind="ExternalInput")
    WL2 = nc.dram_tensor("wl2t", [128, 4, 256], f32, kind="ExternalInput")
    G7 = nc.dram_tensor("g7s", [128, 2], f32, kind="ExternalInput")
    BI2 = nc.dram_tensor("bi2", [128, 2], f32, kind="ExternalInput")
    WL3 = nc.dram_tensor("wl3t", [128, 2, 40], f32, kind="ExternalInput")
    BL3 = nc.dram_tensor("bl3s", [40, 1], f32, kind="ExternalInput")
    OUT = nc.dram_tensor("outq", [40, 1], f32, kind="ExternalOutput")

    with tile.TileContext(nc) as tc:
        with (
            tc.tile_pool(name="const", bufs=1) as constp,
            tc.tile_pool(name="wts", bufs=1) as wts,
            tc.tile_pool(name="feat", bufs=1) as featp,
            tc.tile_pool(name="keysp", bufs=1) as keysp,
            tc.tile_pool(name="work", bufs=1) as work,
            tc.tile_pool(name="work1", bufs=1) as work1,
            tc.tile_pool(name="gat", bufs=2) as gatp,
            tc.tile_pool(name="ps", bufs=2, space="PSUM") as ps,
        ):
            rep_cm = tc.For_i(0, repeat, 1) if repeat > 1 else contextlib.nullcontext()
            with rep_cm:
                # ---------------- constants / weights to SBUF ----------------
                ones1 = constp.tile([1, 512], f32)
                nc.vector.memset(ones1[:], 1.0)
                ones_col = constp.tile([128, 1], f32)
                nc.vector.memset(ones_col[:], 1.0)

                wn_sb, wd_sb, gs_sb, bb_sb = [], [], [], []
                for li, (ci, co) in enumerate(DIMS):
                    nb = (co + 127) // 128
                    cbp = min(co, 128)
                    t_wn = wts.tile([ci, 128 if co == 64 else co], f32, tag=f"wn{li}")
                    t_wd = wts.tile([ci, co], f32, tag=f"wd{li}")
                    t_gs = wts.tile([cbp, nb], f32, tag=f"gs{li}")
                    t_bb = wts.tile([cbp, nb], f32, tag=f"bb{li}")
                    nc.sync.dma_start(t_wn[:], conv_w[li]["wnT"][:])
                    nc.sync.dma_start(t_wd[:], conv_w[li]["wdT"][:])
                    nc.sync.dma_start(t_gs[:], conv_w[li]["gs"][:])
                    nc.sync.dma_start(t_bb[:], conv_w[li]["bb"][:])
                    wn_sb.append(t_wn)
                    wd_sb.append(t_wd)
                    gs_sb.append(t_gs)
                    bb_sb.append(t_bb)

                w5_sb = wts.tile([128, 5, 1024], f32, tag="w5")
                nc.sync.dma_start(w5_sb[:], W5[:])
                g5_sb = wts.tile([128, 8], f32, tag="g5")
                b5_sb = wts.tile([128, 8], f32, tag="b5")
                nc.sync.dma_start(g5_sb[:], G5[:])
                nc.sync.dma_start(b5_sb[:], B5[:])
                wl1_sb = wts.tile([128, 16, 512], f32, tag="wl1")
                nc.sync.dma_start(wl1_sb[:], WL1[:])
                g6_sb = wts.tile([128, 4], f32, tag="g6")
                b6_sb = wts.tile([128, 4], f32, tag="b6")
                nc.sync.dma_start(g6_sb[:], G6[:])
                nc.sync.dma_start(b6_sb[:], B6[:])
                wl2_sb = wts.tile([128, 4, 256], f32, tag="wl2")
                nc.sync.dma_start(wl2_sb[:], WL2[:])
                g7_sb = wts.tile([128, 2], f32, tag="g7")
                bi2_sb = wts.tile([128, 2], f32, tag="bi2")
                nc.sync.dma_start(g7_sb[:], G7[:])
                nc.sync.dma_start(bi2_sb[:], BI2[:])
                wl3_sb = wts.tile([128, 2, 40], f32, tag="wl3")
                nc.sync.dma_start(wl3_sb[:], WL3[:])
                bl3_sb = wts.tile([40, 1], f32, tag="bl3")
                nc.sync.dma_start(bl3_sb[:], BL3[:])

                # input points (layer-0 features), already [C, N]
                pt0 = featp.tile([3, N], f32, tag="ptf3")
                nc.sync.dma_start(pt0[:], X[:])

                # feature tensors for the concat
                f1 = featp.tile([64, N], f32, tag="f1")
                f2 = featp.tile([64, N], f32, tag="f2")
                f3 = featp.tile([128, N], f32, tag="ptf3")
                f4a = featp.tile([128, N], f32, tag="f4a")
                f4b = featp.tile([128, N], f32, tag="f4b")

                def edge_layer(li, PT, out_aps):
                    """PT: AP [Cin, N]. out_aps: list of APs [cb, N] per 128-chan block."""
                    ci, co = DIMS[li]
                    nblk = (co + 127) // 128

                    # ---- squared norms row: nsq = -0.5*|p_m|^2 ----
                    p2 = work.tile([ci, N], f32, tag="p2")
                    nc.scalar.activation(p2[:], PT, AF.Square)
                    psq = ps.tile([1, N], f32, tag="ps")
                    for j in range(4):
                        sl = slice(j * 512, (j + 1) * 512)
                        nc.tensor.matmul(
                            psq[:, sl], ones_col[0:ci, :], p2[:, sl], start=True, stop=True
                        )
                    nsq = work1.tile([1, N], f32, tag="nsq")
                    nc.scalar.activation(nsq[:], psq[:], AF.Copy, scale=-0.5)

                    # ---- distance keys + exact top-24 per row ----
                    # keys'[n,m] = 2 p_n.p_m - |p_m|^2: the -|p_n|^2 row term is a
                    # per-row constant shift, so dropping it leaves top-k unchanged.
                    dec = work1.tile([128, 16, KR], u16, tag="dec")
                    for t in range(16):
                        pg = ps.tile([128, N], f32, tag="ps")
                        tl = slice(t * 128, (t + 1) * 128)
                        for j in range(4):
                            sl = slice(j * 512, (j + 1) * 512)
                            nc.tensor.matmul(
                                pg[:, sl], PT[:, tl], PT[:, sl], start=True, stop=False
                            )
                            nc.tensor.matmul(
                                pg[:, sl], ones1[:, 0:128], nsq[:, sl], start=False, stop=True
                            )
                        keys = keysp.tile([128, N], f16, tag="keys")
                        nc.scalar.activation(keys[:], pg[:], AF.Copy, scale=2.0)
                        for r in range(3):
                            v8 = work.tile([128, 8], f16, tag="v8")
                            nc.vector.max(v8[:], keys[:])
                            nc.vector.max_index(dec[:, t, r * 8 : (r + 1) * 8], v8[:], keys[:])
                            if r < 2:
                                nc.vector.match_replace(keys[:], v8[:], keys[:], -60000.0)

                    # ---- rewrap indices for ap_gather ----
                    # w0[p, g, t, k] = dec[16 g + p, t, k]  (partition fold)
                    rep = co == 64  # replicate channels; split idx halves across cores
                    w0 = work1.tile([16, 8, 16, K], i16, tag="w0")
                    for g in range(8):
                        nc.sync.dma_start(
                            w0[:, g, :, :], dec[16 * g : 16 * (g + 1), :, 0:K].bitcast(i16)
                        )
                    if rep:
                        # cores 0-3 (partitions 0:64) take t in 0..8, cores 4-7 take t in 8..16
                        wrep = work1.tile([128, 8, 8, K], i16, tag="wrep")
                        for h in range(8):
                            th = h // 4
                            nc.sync.dma_start(
                                wrep[16 * h : 16 * (h + 1), :, :, :],
                                w0[:, :, th * 8 : (th + 1) * 8, :],
                            )
                    else:
                        wrep = work1.tile([128, 8, 16, K], i16, tag="wrep")
                        for h in range(8):
                            nc.sync.dma_start(wrep[16 * h : 16 * (h + 1), :, :, :], w0[:])

                    # ---- per channel-block: Y/c matmuls, gather+max, bn+relu ----
                    if nblk == 2:
                        # l4: pack both 128-ch blocks as d=2 -> half the gather index-ops
                        yt_pk = work1.tile([128, N, 2], f32, tag="ytpk")
                        for blk in range(2):
                            pym = ps.tile([128, N], f32, tag="ps")
                            for j in range(4):
                                sl = slice(j * 512, (j + 1) * 512)
                                nc.tensor.matmul(
                                    pym[:, sl],
                                    wn_sb[li][:, blk * 128 : blk * 128 + 128],
                                    PT[:, sl],
                                    start=True,
                                    stop=True,
                                )
                            nc.scalar.activation(yt_pk[:, :, blk], pym[:], AF.Copy)
                        mts = []
                        for blk in range(2):
                            sfx = "" if blk == 0 else "b"
                            mt_t = work1.tile(
                                [128, N], f32, tag="mt" + sfx, name=f"mt4_{blk}"
                            )
                            mts.append(mt_t)
                        mt4s = [
                            m[:].rearrange("c (t g p) -> c t g p", t=16, g=8, p=16)
                            for m in mts
                        ]
                        for g in range(8):
                            for q in range(4):
                                gt = gatp.tile([128, 4, K, 16, 2], f32, tag="gath")
                                idxs = wrep[0:128, g, q * 4 : (q + 1) * 4, :]
                                nc.gpsimd.ap_gather(
                                    gt[:],
                                    yt_pk[:],
                                    idxs,
                                    channels=128,
                                    num_elems=N,
                                    d=2,
                                    num_idxs=4 * K * 16,
                                )
                                for blk in range(2):
                                    nc.vector.tensor_reduce(
                                        mt4s[blk][:, q * 4 : (q + 1) * 4, g, :],
                                        gt[:, :, :, :, blk].transpose([0, 1, 3, 2]),
                                        axis=AX.X,
                                        op=ALU.max,
                                    )
                        for blk in range(2):
                            pcm = ps.tile([128, N], f32, tag="ps")
                            for j in range(4):
                                sl = slice(j * 512, (j + 1) * 512)
                                nc.tensor.matmul(
                                    pcm[:, sl],
                                    wd_sb[li][:, blk * 128 : blk * 128 + 128],
                                    PT[:, sl],
                                    start=True,
                                    stop=True,
                                )
                            ct = work1.tile([128, N], f32, tag="ct")
                            nc.scalar.activation(ct[:], pcm[:], AF.Copy)
                            mt = mts[blk]
                            nc.vector.tensor_tensor(mt[:], mt[:], ct[:], ALU.add)
                            nc.scalar.activation(
                                out_aps[blk],
                                mt[:],
                                AF.Relu,
                                bias=bb_sb[li][0:128, blk : blk + 1],
                                scale=gs_sb[li][0:128, blk : blk + 1],
                            )
                        return
                    for blk in range(nblk):
                        cb = min(co - blk * 128, 128)
                        csl = slice(blk * 128, blk * 128 + cb)
                        gcb = 128 if rep else cb  # gather-channel count (replicated rows)
                        ysl = slice(blk * 128, blk * 128 + gcb)
                        yt = work1.tile([gcb, N], f32, tag="yt")
                        ct = work1.tile([cb, N], f32, tag="ct")
                        pym = ps.tile([gcb, N], f32, tag="ps")
                        for j in range(4):
                            sl = slice(j * 512, (j + 1) * 512)
                            nc.tensor.matmul(
                                pym[:, sl], wn_sb[li][:, ysl], PT[:, sl], start=True, stop=True
                            )
                        nc.scalar.activation(yt[:], pym[:], AF.Copy)
                        pcm = ps.tile([cb, N], f32, tag="ps")
                        for j in range(4):
                            sl = slice(j * 512, (j + 1) * 512)
                            nc.tensor.matmul(
                                pcm[:, sl], wd_sb[li][:, csl], PT[:, sl], start=True, stop=True
                            )
                        nc.scalar.activation(ct[:], pcm[:], AF.Copy)

                        mt = work1.tile([cb, N], f32, tag="mt")
                        mt4 = mt[:].rearrange("c (t g p) -> c t g p", t=16, g=8, p=16)
                        if rep:
                            # 8 calls; replica halves carry different t-halves
                            for g in range(8):
                                gt = gatp.tile([128, 8, K, 16], f32, tag="gath")
                                idxs = wrep[:, g, :, :]
                                nc.gpsimd.ap_gather(
                                    gt[:],
                                    yt[:],
                                    idxs,
                                    channels=128,
                                    num_elems=N,
                                    d=1,
                                    num_idxs=8 * K * 16,
                                )
                                for th in range(2):
                                    nc.vector.tensor_reduce(
                                        mt4[:, th * 8 : (th + 1) * 8, g, :],
                                        gt[64 * th : 64 * th + 64].transpose([0, 1, 3, 2]),
                                        axis=AX.X,
                                        op=ALU.max,
                                    )
                        else:
                            for g in range(8):
                                for th in range(2):
                                    gt = gatp.tile([cb, 8, K, 16], f32, tag="gath")
                                    idxs = wrep[0:cb, g, th * 8 : (th + 1) * 8, :]
                                    nc.gpsimd.ap_gather(
                                        gt[:],
                                        yt[:],
                                        idxs,
                                        channels=cb,
                                        num_elems=N,
                                        d=1,
                                        num_idxs=8 * K * 16,
                                    )
                                    nc.vector.tensor_reduce(
                                        mt4[:, th * 8 : (th + 1) * 8, g, :],
                                        gt[:].transpose([0, 1, 3, 2]),
                                        axis=AX.X,
                                        op=ALU.max,
                                    )
                        # center term + bn + relu
                        nc.vector.tensor_tensor(mt[:], mt[:], ct[:], ALU.add)
                        nc.scalar.activation(
                            out_aps[blk],
                            mt[:],
                            AF.Relu,
                            bias=bb_sb[li][0:cb, blk : blk + 1],
                            scale=gs_sb[li][0:cb, blk : blk + 1],
                        )

                edge_layer(0, pt0[:], [f1[:]])
                edge_layer(1, f1[:], [f2[:]])
                edge_layer(2, f2[:], [f3[:]])
                edge_layer(3, f3[:], [f4a[:], f4b[:]])

                # ---------------- conv5 (1024) + global max/mean pool ----------------
                pooled = work1.tile([128, 16], f32, tag="pooled")
                rhs_chunks = [f1[:], f2[:], f3[:], f4a[:], f4b[:]]
                chunk_rows = [64, 64, 128, 128, 128]
                for blk in range(8):
                    bsl = slice(blk * 128, (blk + 1) * 128)
                    ph = ps.tile([128, N], f32, tag="ps")
                    for j in range(4):
                        sl = slice(j * 512, (j + 1) * 512)
                        for c in range(5):
                            nc.tensor.matmul(
                                ph[:, sl],
                                w5_sb[0 : chunk_rows[c], c, bsl],
                                rhs_chunks[c][:, sl],
                                start=(c == 0),
                                stop=(c == 4),
                            )
                    hb = work.tile([128, N], f32, tag="p2")  # share big-scratch slots
                    nc.scalar.activation(
                        hb[:],
                        ph[:],
                        AF.Relu,
                        bias=b5_sb[:, blk : blk + 1],
                        scale=g5_sb[:, blk : blk + 1],
                        accum_out=pooled[:, 8 + blk : 9 + blk],
                    )
                    nc.vector.tensor_reduce(
                        pooled[:, blk : blk + 1], hb[:], axis=AX.X, op=ALU.max
                    )

                # ---------------- MLP head ----------------
                ps1 = ps.tile([128, 4], f32, tag="ps")
                for mb in range(4):
                    for c in range(16):
                        nc.tensor.matmul(
                            ps1[:, mb : mb + 1],
                            wl1_sb[:, c, mb * 128 : (mb + 1) * 128],
                            pooled[:, c : c + 1],
                            start=(c == 0),
                            stop=(c == 15),
                        )
                s1 = work1.tile([128, 4], f32, tag="s1")
                s1p = work1.tile([128, 4], f32, tag="s1p")
                for mb in range(4):
                    nc.scalar.activation(
                        s1p[:, mb : mb + 1],
                        ps1[:, mb : mb + 1],
                        AF.Identity,
                        bias=b6_sb[:, mb : mb + 1],
                        scale=g6_sb[:, mb : mb + 1],
                    )
                nc.vector.scalar_tensor_tensor(
                    s1[:], s1p[:], 0.2, s1p[:], op0=ALU.mult, op1=ALU.max
                )
                ps2 = ps.tile([128, 2], f32, tag="ps")
                for mb in range(2):
                    for c in range(4):
                        nc.tensor.matmul(
                            ps2[:, mb : mb + 1],
                            wl2_sb[:, c, mb * 128 : (mb + 1) * 128],
                            s1[:, c : c + 1],
                            start=(c == 0),
                            stop=(c == 3),
                        )
                s2 = work1.tile([128, 2], f32, tag="s2")
                s2p = work1.tile([128, 2], f32, tag="s2p")
                for mb in range(2):
                    nc.scalar.activation(
                        s2p[:, mb : mb + 1],
                        ps2[:, mb : mb + 1],
                        AF.Identity,
                        bias=bi2_sb[:, mb : mb + 1],
                        scale=g7_sb[:, mb : mb + 1],
                    )
                nc.vector.scalar_tensor_tensor(
                    s2[:], s2p[:], 0.2, s2p[:], op0=ALU.mult, op1=ALU.max
                )
                ps3 = ps.tile([40, 1], f32, tag="ps")
                for c in range(2):
                    nc.tensor.matmul(
                        ps3[:],
                        wl3_sb[0:128, c, :],
                        s2[:, c : c + 1],
                        start=(c == 0),
                        stop=(c == 1),
                    )
                osb = work1.tile([40, 1], f32, tag="osb")
                nc.vector.tensor_tensor(osb[:], ps3[:], bl3_sb[:], ALU.add)
                nc.sync.dma_start(OUT[:], osb[:])

    nc.compile()
    return nc


def _get_module():
    if "nc" not in _CACHE:
        _CACHE["nc"] = _build_module()
    return _CACHE["nc"]


def _prep_weights(inp):
    """Host-side weight preprocessing -> dict of DRAM tensor arrays (fp32)."""
    f = np.float32
    out = {}
    ws = [
        (inp["w1"], inp["g1"], inp["b1"]),
        (inp["w2"], inp["g2"], inp["b2"]),
        (inp["w3"], inp["g3"], inp["b3"]),
        (inp["w4"], inp["g4"], inp["b4"]),
    ]
    for li, ((w, g, b), (ci, co)) in enumerate(zip(ws, DIMS)):
        w = np.asarray(w, f)
        nb = (co + 127) // 128
        cb = min(co, 128)
        wnT = w[:, :ci].T
        if co == 64:
            wnT = np.concatenate([wnT, wnT], axis=1)  # duplicate rows of Y
        out[f"wn{li}"] = np.ascontiguousarray(wnT)
        out[f"wd{li}"] = np.ascontiguousarray((w[:, ci:] - w[:, :ci]).T)
        out[f"gs{li}"] = np.ascontiguousarray(
            (np.asarray(g, f) * BN_SCALE).reshape(nb, cb).T
        )
        out[f"bb{li}"] = np.ascontiguousarray(np.asarray(b, f).reshape(nb, cb).T)
    w5 = np.asarray(inp["w5"], f)  # [1024, 512]
    w5t = w5.T  # [512, 1024]
    w5t_rs = np.zeros((128, 5, 1024), f)
    w5t_rs[0:64, 0, :] = w5t[0:64]
    w5t_rs[0:64, 1, :] = w5t[64:128]
    w5t_rs[:, 2, :] = w5t[128:256]
    w5t_rs[:, 3, :] = w5t[256:384]
    w5t_rs[:, 4, :] = w5t[384:512]
    out["w5t"] = w5t_rs
    out["g5s"] = np.ascontiguousarray(
        (np.asarray(inp["g5"], f) * BN_SCALE).reshape(8, 128).T
    )
    out["b5s"] = np.ascontiguousarray(np.asarray(inp["b5"], f).reshape(8, 128).T)
    wl1 = np.asarray(inp["wl1"], f).copy()  # [512, 2048]
    wl1[:, 1024:] *= f(1.0 / N)  # fold the mean-pool division
    out["wl1t"] = np.ascontiguousarray(wl1.T.reshape(16, 128, 512).transpose(1, 0, 2))
    out["g6s"] = np.ascontiguousarray(
        (np.asarray(inp["g6"], f) * BN_SCALE).reshape(4, 128).T
    )
    out["b6s"] = np.ascontiguousarray(np.asarray(inp["b6"], f).reshape(4, 128).T)
    wl2 = np.asarray(inp["wl2"], f)  # [256, 512]
    out["wl2t"] = np.ascontiguousarray(wl2.T.reshape(4, 128, 256).transpose(1, 0, 2))
    g7s = np.asarray(inp["g7"], f) * BN_SCALE
    out["g7s"] = np.ascontiguousarray(g7s.reshape(2, 128).T)
    bi2 = np.asarray(inp["bl2"], f) * g7s + np.asarray(inp["b7"], f)
    out["bi2"] = np.ascontiguousarray(bi2.reshape(2, 128).T)
    wl3 = np.asarray(inp["wl3"], f)  # [40, 256]
    out["wl3t"] = np.ascontiguousarray(wl3.T.reshape(2, 128, 40).transpose(1, 0, 2))
    out["bl3s"] = np.ascontiguousarray(np.asarray(inp["bl3"], f).reshape(40, 1))
    return out


def kernel(**inputs):
    from concourse.bass_utils import run_bass_kernel_spmd

    nc = _get_module()
    wmap = _prep_weights(inputs)
    x = np.asarray(inputs["x"], np.float32)  # [8, 3, 2048]
    in_maps = []
    for c in range(NCORES):
        m = dict(wmap)
        m["xq"] = np.ascontiguousarray(x[c])
        in_maps.append(m)
    res = run_bass_kernel_spmd(nc, in_maps, core_ids=list(range(NCORES)))
    out = np.stack([res.results[c]["outq"].reshape(40) for c in range(NCORES)])
    return out.astype(np.float32)


if __name__ == "__main__":
    nc = _get_module()
    print("module built OK")



# revision 45
# speedup vs baseline: 2.1561x; 2.1561x over previous
"""DGCNN kernel for Trainium2 (Bass/Tile), data-parallel over batch across 8 cores.

Key algorithmic decomposition (per core, one point cloud of N=2048 points):
  EdgeConv(max_k relu(bn(W @ [nb - ctr; ctr]))) decomposes as
    h[n,k,o] = Wn·p_{idx(n,k)} + (Wc - Wn)·p_n         (Wn = W[:, :C], Wc = W[:, C:])
  and since BN scale > 0 and relu is monotonic,
    out[n] = relu(bn(max_k (Wn·p_idx) + Wd·p_n))       (Wd = Wc - Wn)
  So per layer: Y = P @ WnT (one matmul over points), gather+max over the 20
  kNN rows of Y, add the center term, bn+relu.

  kNN: keys[n,m] = 2*(p_n·p_m) - |p_n|^2 - |p_m|^2 (= -d2), built fully on the
  PE via rank-1 correction matmuls. Top-24 per row via "packed keys": the
  PSUM keys are affinely mapped into [1088, 6955] and rounded to f16 (integer-
  valued there), then packed = kq*2048 + m encodes (quantized key, column) in
  one exact f32; 3 rounds of max8/match_replace extract the top-24 and a
  single mod-2048 recovers the indices (no max_index passes). Quantization is
  ~10-12 bits on the key range, comparable to the f16 keys it replaces; index
  tiebreak is deterministic. All large matmuls run as float32r (4x PE rate).

  Gather runs on GPSIMD (ap_gather) against Y^T [Cout, N]; the index list is
  rewrapped to the per-core [16, S] partition-interleaved layout with two
  small SBUF->SBUF DMA steps (partition fold + replicate).
"""

import sys

import numpy as np

sys.path.insert(0, "/opt/trn_rl_repo")

EPS = 1e-5
BN_SCALE = np.float32(1.0 / np.sqrt(1.0 + EPS))
N = 2048
K = 20
KR = 24  # extracted per row (3 rounds of max8)
NCORES = 8
DIMS = [(3, 64), (64, 64), (64, 128), (128, 256)]  # (Cin, Cout) per edge conv
# per-layer gather packing: (d, partitions per replica, g-groups per call)
GATH = [(2, 32, 4), (2, 32, 4), (4, 32, 2), (4, 64, 2)]

_CACHE = {}


def _build_module(repeat=1):
    import contextlib
    import os

    ablate = os.environ.get("ABLATE", "")

    from concourse import bacc, mybir, tile

    dt = mybir.dt
    f32 = dt.float32
    f32r = dt.float32r
    f16 = dt.float16
    u16 = dt.uint16
    i16 = dt.int16
    i32 = dt.int32
    AF = mybir.ActivationFunctionType
    ALU = mybir.AluOpType
    AX = mybir.AxisListType

    def r(ap):
        return ap.bitcast(f32r)

    nc = bacc.Bacc("TRN2", target_bir_lowering=False, debug=False)

    # ---------------- DRAM I/O ----------------
    X = nc.dram_tensor("xq", [3, N], f32r, kind="ExternalInput")
    conv_w = []
    for li, (ci, co) in enumerate(DIMS):
        nb = (co + 127) // 128
        cb = min(co, 128)
        conv_w.append(
            dict(
                wnT=nc.dram_tensor(
                    f"wn{li}", [ci, 128 * GATH[li][0]], f32r, kind="ExternalInput"
                ),
                wdT=nc.dram_tensor(f"wd{li}", [ci, co], f32r, kind="ExternalInput"),
                gs=nc.dram_tensor(f"gs{li}", [cb, nb], f32, kind="ExternalInput"),
                bb=nc.dram_tensor(f"bb{li}", [cb, nb], f32, kind="ExternalInput"),
            )
        )
    W5 = nc.dram_tensor("w5t", [128, 5, 1024], f32r, kind="ExternalInput")
    G5 = nc.dram_tensor("g5s", [128, 8], f32, kind="ExternalInput")
    B5 = nc.dram_tensor("b5s", [128, 8], f32, kind="ExternalInput")
    WL1 = nc.dram_tensor("wl1t", [128, 16, 512], f32, kind="ExternalInput")
    G6 = nc.dram_tensor("g6s", [128, 4], f32, kind="ExternalInput")
    B6 = nc.dram_tensor("b6s", [128, 4], f32, kind="ExternalInput")
    WL2 = nc.dram_tensor("wl2t", [128, 4, 256], f32, kind="ExternalInput")
    G7 = nc.dram_tensor("g7s", [128, 2], f32, kind="ExternalInput")
    BI2 = nc.dram_tensor("bi2", [128, 2], f32, kind="ExternalInput")
    WL3 = nc.dram_tensor("wl3t", [128, 2, 40], f32, kind="ExternalInput")
    BL3 = nc.dram_tensor("bl3s", [40, 1], f32, kind="ExternalInput")
    OUT = nc.dram_tensor("outq", [40, 1], f32, kind="ExternalOutput")

    with tile.TileContext(nc) as tc:
        with (
            tc.tile_pool(name="const", bufs=1) as constp,
            tc.tile_pool(name="wts", bufs=1) as wts,
            tc.tile_pool(name="feat", bufs=1) as featp,
            tc.tile_pool(name="pkp", bufs=2) as pkp,
            tc.tile_pool(name="work", bufs=1) as work,
            tc.tile_pool(name="work1", bufs=1) as work1,
            tc.tile_pool(name="gat", bufs=4) as gatp,
            tc.tile_pool(name="idxp", bufs=2) as idxp,
            tc.tile_pool(name="ps", bufs=2, space="PSUM") as ps,
        ):
            rep_cm = tc.For_i(0, repeat, 1) if repeat > 1 else contextlib.nullcontext()
            with rep_cm:
                # ---------------- constants / weights to SBUF ----------------
                ones1 = constp.tile([1, 512], f32r)
                ones_col = constp.tile([128, 1], f32r)
                tmpf = work1.tile([128, 512], f32, tag="ct")
                nc.vector.memset(tmpf[:], 1.0)
                nc.scalar.activation(ones1[:], tmpf[0:1, :], AF.Copy)
                nc.scalar.activation(ones_col[:], tmpf[:, 0:1], AF.Copy)
                # packed-offset vector: mvec2[m] = 8102*2048 + m  (f32-exact)
                mvec_tmp = work1.tile([128, N], u16, tag="yt")
                nc.gpsimd.iota(mvec_tmp[:], pattern=[[1, N]], channel_multiplier=0)
                mvec2 = constp.tile([128, N], f32)
                nc.vector.tensor_scalar(
                    mvec2[:], mvec_tmp[:], float(8102 * 2048), None, op0=ALU.add
                )

                wn_sb, wd_sb, gs_sb, bb_sb = [], [], [], []
                for li, (ci, co) in enumerate(DIMS):
                    nb = (co + 127) // 128
                    cbp = min(co, 128)
                    t_wn = wts.tile([ci, 128 * GATH[li][0]], f32r, tag=f"wn{li}")
                    t_wd = wts.tile([ci, co], f32r, tag=f"wd{li}")
                    t_gs = wts.tile([cbp, nb], f32, tag=f"gs{li}")
                    t_bb = wts.tile([cbp, nb], f32, tag=f"bb{li}")
                    nc.sync.dma_start(t_wn[:], conv_w[li]["wnT"][:])
                    nc.sync.dma_start(t_wd[:], conv_w[li]["wdT"][:])
                    nc.sync.dma_start(t_gs[:], conv_w[li]["gs"][:])
                    nc.sync.dma_start(t_bb[:], conv_w[li]["bb"][:])
                    wn_sb.append(t_wn)
                    wd_sb.append(t_wd)
                    gs_sb.append(t_gs)
                    bb_sb.append(t_bb)

                w5_sb = wts.tile([128, 5, 1024], f32r, tag="w5")
                nc.sync.dma_start(w5_sb[:], W5[:])
                g5_sb = wts.tile([128, 8], f32, tag="g5")
                b5_sb = wts.tile([128, 8], f32, tag="b5")
                nc.sync.dma_start(g5_sb[:], G5[:])
                nc.sync.dma_start(b5_sb[:], B5[:])
                wl1_sb = wts.tile([128, 16, 512], f32, tag="wl1")
                nc.sync.dma_start(wl1_sb[:], WL1[:])
                g6_sb = wts.tile([128, 4], f32, tag="g6")
                b6_sb = wts.tile([128, 4], f32, tag="b6")
                nc.sync.dma_start(g6_sb[:], G6[:])
                nc.sync.dma_start(b6_sb[:], B6[:])
                wl2_sb = wts.tile([128, 4, 256], f32, tag="wl2")
                nc.sync.dma_start(wl2_sb[:], WL2[:])
                g7_sb = wts.tile([128, 2], f32, tag="g7")
                bi2_sb = wts.tile([128, 2], f32, tag="bi2")
                nc.sync.dma_start(g7_sb[:], G7[:])
                nc.sync.dma_start(bi2_sb[:], BI2[:])
                wl3_sb = wts.tile([128, 2, 40], f32, tag="wl3")
                nc.sync.dma_start(wl3_sb[:], WL3[:])
                bl3_sb = wts.tile([40, 1], f32, tag="bl3")
                nc.sync.dma_start(bl3_sb[:], BL3[:])

                # input points (layer-0 features), already [C, N]
                pt0 = featp.tile([3, N], f32r, tag="ptf3")
                nc.sync.dma_start(pt0[:], X[:])

                # feature tensors for the concat
                f1 = featp.tile([64, N], f32r, tag="f1")
                f2 = featp.tile([64, N], f32r, tag="f2")
                f3 = featp.tile([128, N], f32r, tag="ptf3")
                f4a = featp.tile([128, N], f32r, tag="f4a")
                f4b = featp.tile([128, N], f32r, tag="f4b")

                def edge_layer(li, PT, out_aps):
                    """PT: AP [Cin, N]. out_aps: list of APs [cb, N] per 128-chan block."""
                    ci, co = DIMS[li]
                    nblk = (co + 127) // 128

                    # ---- squared norms row: nsq = -0.5*|p_m|^2 ----
                    p2 = work.tile([ci, N], f32r, tag="p2")
                    nc.scalar.activation(p2[:], PT, AF.Square)
                    psq = ps.tile([1, N], f32, tag="ps")
                    for j in range(4):
                        sl = slice(j * 512, (j + 1) * 512)
                        nc.tensor.matmul(
                            psq[:, sl], r(ones_col[0:ci, :]), r(p2[:, sl]),
                            start=True, stop=True,
                        )
                    nsq = work1.tile([1, N], f32r, tag="ct")
                    nc.scalar.activation(nsq[:], psq[:], AF.Copy, scale=-0.5)

                    # sqrt-compressed 13-bit quantization scale:
                    # vq = rint(sqrt(s2^2*d2 + 4096)), s2 = 8100/sqrt(4*M2+1),
                    # d2 = |p_n|^2 - 2*pg  (pg includes the -0.5|q|^2 term)
                    m2t = work1.tile([1, 1], f32, tag="m2t")
                    nc.vector.tensor_reduce(m2t[:], psq[:], axis=AX.X, op=ALU.max)
                    t1 = work1.tile([1, 1], f32, tag="t1")
                    nc.vector.tensor_scalar(t1[:], m2t[:], 4.0, 1.0, op0=ALU.mult, op1=ALU.add)
                    t2 = work1.tile([1, 1], f32, tag="t2")
                    nc.scalar.sqrt(t2[:], t1[:])
                    rec = work1.tile([1, 1], f32, tag="srec")
                    nc.vector.reciprocal(rec[:], t2[:])
                    s2t = work1.tile([1, 1], f32, tag="s2t")
                    nc.vector.tensor_scalar(s2t[:], rec[:], 8100.0, None, op0=ALU.mult)
                    s2sq = work1.tile([1, 1], f32, tag="s2sq")
                    nc.vector.tensor_tensor(s2sq[:], s2t[:], s2t[:], ALU.mult)
                    scA = work1.tile([1, 1], f32, tag="scA")
                    nc.vector.tensor_scalar(scA[:], s2sq[:], -2.0, None, op0=ALU.mult)
                    scA_bc = work1.tile([128, 1], f32, tag="scAbc")
                    nc.gpsimd.partition_broadcast(scA_bc[:], scA[:])
                    s2sq_bc = work1.tile([128, 1], f32, tag="s2sqbc")
                    nc.gpsimd.partition_broadcast(s2sq_bc[:], s2sq[:])
                    # row squared-norms transposed to [128, 16] and scaled:
                    # rqs[p, t] = s2^2*|p_(t*128+p)|^2 + 4096 (floor masks f32r noise)
                    rqt = work1.tile([128, 16], f32, tag="rqt")
                    for t in range(16):
                        nc.sync.dma_start(
                            rqt[:, t : t + 1],
                            nsq[0:1, t * 128 : (t + 1) * 128].bitcast(f32),
                        )
                    rqs = work1.tile([128, 16], f32, tag="rqs")
                    nc.scalar.activation(rqs[:], rqt[:], AF.Copy, scale=scA_bc[:], bias=4096.0)

                    # ---- Y matmuls into packed/replicated f16 layout ----
                    # (issued first: needs only PT, so PE/Act fill early and
                    # the first round's gathers can start immediately)
                    d_, pp, gb = GATH[li]
                    R = 128 // pp
                    tgrp = 16 // R
                    ncall = 8 // gb  # gather calls per round
                    yt_pk = work1.tile([128, N, d_], f16, tag="yt")
                    for dd in range(d_):
                        pym = ps.tile([128, N], f32, tag="ps")
                        for j in range(4):
                            sl = slice(j * 512, (j + 1) * 512)
                            nc.tensor.matmul(
                                pym[:, sl],
                                wn_sb[li][:, dd * 128 : (dd + 1) * 128],
                                PT[:, sl],
                                start=True,
                                stop=True,
                            )
                        nc.scalar.activation(yt_pk[:, :, dd], pym[:], AF.Copy)

                    mts = []
                    for blk in range(nblk):
                        cb = min(co - blk * 128, 128)
                        mt_t = work1.tile(
                            [cb, N], f32, tag="mt" + ("b" if blk else ""),
                            name=f"mt_{li}_{blk}",
                        )
                        mts.append(mt_t)
                    mt4s = [
                        m[:].rearrange("c (t g p) -> c t g p", t=16, g=8, p=16)
                        for m in mts
                    ]
                    if ablate == "gather":
                        for m in mts:
                            nc.vector.memset(m[:], 0.0)

                    def issue_reduces(rho, gts):
                        """K-max reduces for round rho's gather outputs."""
                        for c2, gt in gts:
                            for r_ in range(R):
                                t = r_ * tgrp + rho
                                for dd in range(d_):
                                    ch0 = dd * pp
                                    nc.vector.tensor_reduce(
                                        mt4s[ch0 // 128][
                                            ch0 % 128 : ch0 % 128 + pp,
                                            t,
                                            c2 * gb : (c2 + 1) * gb,
                                            :,
                                        ],
                                        gt[r_ * pp : (r_ + 1) * pp, :, :, :, dd]
                                        .transpose([0, 1, 3, 2]),
                                        axis=AX.X,
                                        op=ALU.max,
                                    )

                    # ---- top-k rounds, round-robin over replica groups ----
                    # Round rho does top-k for tiles {r*tgrp + rho}. Its gathers
                    # (Pool) are issued right away and overlap the NEXT round's
                    # top-k on the DVE; the reduces for round rho are issued
                    # interleaved between round rho+1's tiles so the in-order
                    # DVE queue never stalls on the Pool engine.
                    va = work1.tile([128, 16, KR], f32, tag="va")
                    pend = []  # (rho, [(c2, gt), ...]) awaiting reduces
                    for rho in range(tgrp):
                        for r_ in range(R):
                            t = r_ * tgrp + rho
                            pg = ps.tile([128, N], f32, tag="ps")
                            tl = slice(t * 128, (t + 1) * 128)
                            for j in range(4):
                                sl = slice(j * 512, (j + 1) * 512)
                                nc.tensor.matmul(
                                    pg[:, sl], PT[:, tl], PT[:, sl],
                                    start=True, stop=False,
                                )
                                nc.tensor.matmul(
                                    pg[:, sl], ones1[:, 0:128], nsq[:, sl],
                                    start=False, stop=True,
                                )
                            packed = pkp.tile([128, N], f32, tag="pk")
                            nc.scalar.activation(
                                packed[:].bitcast(i32), pg[:], AF.Sqrt,
                                scale=scA_bc[:], bias=rqs[:, t : t + 1],
                            )
                            nc.vector.scalar_tensor_tensor(
                                packed[:], packed[:].bitcast(i32), -2048.0,
                                mvec2[:], op0=ALU.mult, op1=ALU.add,
                            )
                            if ablate == "topk":
                                nc.vector.memset(va[:, t, :], 16590848.0)
                            else:
                                for rr in range(3):
                                    nc.vector.max(
                                        va[:, t, rr * 8 : (rr + 1) * 8], packed[:]
                                    )
                                    if rr < 2:
                                        nc.vector.match_replace(
                                            packed[:],
                                            va[:, t, rr * 8 : (rr + 1) * 8],
                                            packed[:], 0.0,
                                        )
                        # indices of this round: m = packed & 2047 (exact ints)
                        vir = idxp.tile([128, R, K], i32, tag="vir")
                        nc.vector.tensor_copy(vir[:, :, :], va[:, rho::tgrp, 0:K])
                        nc.vector.tensor_scalar(
                            vir[:, :, :], vir[:, :, :], 2047, None,
                            op0=ALU.bitwise_and,
                        )
                        decr = idxp.tile([128, R, K], i16, tag="decr")
                        nc.vector.tensor_copy(decr[:, :, :], vir[:, :, :])
                        # fold + per-core rewrap for this round
                        w0r = idxp.tile([16, 8, R, K], i16, tag="w0r")
                        for g in range(8):
                            nc.sync.dma_start(
                                w0r[:, g, :, :], decr[16 * g : 16 * (g + 1), :, :]
                            )
                        wrepr = idxp.tile([128, 8, K], i16, tag="wrepr")
                        for h in range(8):
                            rr_ = h // (8 // R)
                            nc.sync.dma_start(
                                wrepr[16 * h : 16 * (h + 1), :, :],
                                w0r[:, :, rr_, :],
                            )
                        if ablate != "gather":
                            gts = []
                            for c2 in range(ncall):
                                gt = gatp.tile(
                                    [128, gb, K, 16, d_], f16, tag="gath"
                                )
                                nc.gpsimd.ap_gather(
                                    gt[:],
                                    yt_pk[:],
                                    wrepr[:, c2 * gb : (c2 + 1) * gb, :],
                                    channels=128,
                                    num_elems=N,
                                    d=d_,
                                    num_idxs=gb * K * 16,
                                )
                                gts.append((c2, gt))
                            pend.append((rho, gts))
                        # previous round's reduces go behind this round's
                        # extract so the gathers of round rho are already in
                        # flight before the DVE waits on round rho-1's data
                        if len(pend) > 1:
                            prho, gts = pend.pop(0)
                            issue_reduces(prho, gts)
                    for prho, gts in pend:
                        issue_reduces(prho, gts)

                    # ---- center term + bn + relu per 128-chan block ----
                    for blk in range(nblk):
                        cb = min(co - blk * 128, 128)
                        csl = slice(blk * 128, blk * 128 + cb)
                        pcm = ps.tile([cb, N], f32, tag="ps")
                        for j in range(4):
                            sl = slice(j * 512, (j + 1) * 512)
                            nc.tensor.matmul(
                                pcm[:, sl], wd_sb[li][:, csl], PT[:, sl],
                                start=True, stop=True,
                            )
                        ct = work1.tile([cb, N], f32, tag="ct")
                        nc.scalar.activation(ct[:], pcm[:], AF.Copy)
                        mt = mts[blk]
                        nc.vector.tensor_tensor(mt[:], mt[:], ct[:], ALU.add)
                        nc.scalar.activation(
                            out_aps[blk],
                            mt[:],
                            AF.Relu,
                            bias=bb_sb[li][0:cb, blk : blk + 1],
                            scale=gs_sb[li][0:cb, blk : blk + 1],
                        )

                edge_layer(0, pt0[:], [f1[:]])
                edge_layer(1, f1[:], [f2[:]])
                edge_layer(2, f2[:], [f3[:]])
                edge_layer(3, f3[:], [f4a[:], f4b[:]])

                # ---------------- conv5 (1024) + global max/mean pool ----------------
                pooled = work1.tile([128, 16], f32, tag="pooled")
                rhs_chunks = [f1[:], f2[:], f3[:], f4a[:], f4b[:]]
                chunk_rows = [64, 64, 128, 128, 128]
                for blk in range(8):
                    bsl = slice(blk * 128, (blk + 1) * 128)
                    ph = ps.tile([128, N], f32, tag="ps")
                    for j in range(4):
                        sl = slice(j * 512, (j + 1) * 512)
                        for c in range(5):
                            nc.tensor.matmul(
                                ph[:, sl],
                                r(w5_sb[0 : chunk_rows[c], c, bsl]),
                                r(rhs_chunks[c][:, sl]),
                                start=(c == 0),
                                stop=(c == 4),
                            )
                    hb = work.tile([128, N], f32, tag="p2")  # share big-scratch slots
                    nc.scalar.activation(
                        hb[:],
                        ph[:],
                        AF.Relu,
                        bias=b5_sb[:, blk : blk + 1],
                        scale=g5_sb[:, blk : blk + 1],
                        accum_out=pooled[:, 8 + blk : 9 + blk],
                    )
                    nc.vector.tensor_reduce(
                        pooled[:, blk : blk + 1], hb[:], axis=AX.X, op=ALU.max
                    )

                # ---------------- MLP head ----------------
                ps1 = ps.tile([128, 4], f32, tag="ps")
                for mb in range(4):
                    for c in range(16):
                        nc.tensor.matmul(
                            ps1[:, mb : mb + 1],
                            wl1_sb[:, c, mb * 128 : (mb + 1) * 128],
                            pooled[:, c : c + 1],
                            start=(c == 0),
                            stop=(c == 15),
                        )
                s1 = work1.tile([128, 4], f32, tag="s1")
                s1p = work1.tile([128, 4], f32, tag="s1p")
                for mb in range(4):
                    nc.scalar.activation(
                        s1p[:, mb : mb + 1],
                        ps1[:, mb : mb + 1],
                        AF.Identity,
                        bias=b6_sb[:, mb : mb + 1],
                        scale=g6_sb[:, mb : mb + 1],
                    )
                nc.vector.scalar_tensor_tensor(
                    s1[:], s1p[:], 0.2, s1p[:], op0=ALU.mult, op1=ALU.max
                )
                ps2 = ps.tile([128, 2], f32, tag="ps")
                for mb in range(2):
                    for c in range(4):
                        nc.tensor.matmul(
                            ps2[:, mb : mb + 1],
                            wl2_sb[:, c, mb * 128 : (mb + 1) * 128],
                            s1[:, c : c + 1],
                            start=(c == 0),
                            stop=(c == 3),
                        )
                s2 = work1.tile([128, 2], f32, tag="s2")
                s2p = work1.tile([128, 2], f32, tag="s2p")
                for mb in range(2):
                    nc.scalar.activation(
                        s2p[:, mb : mb + 1],
                        ps2[:, mb : mb + 1],
                        AF.Identity,
                        bias=bi2_sb[:, mb : mb + 1],
                        scale=g7_sb[:, mb : mb + 1],
                    )
                nc.vector.scalar_tensor_tensor(
                    s2[:], s2p[:], 0.2, s2p[:], op0=ALU.mult, op1=ALU.max
                )
                ps3 = ps.tile([40, 1], f32, tag="ps")
                for c in range(2):
                    nc.tensor.matmul(
                        ps3[:],
                        wl3_sb[0:128, c, :],
                        s2[:, c : c + 1],
                        start=(c == 0),
                        stop=(c == 1),
                    )
                osb = work1.tile([40, 1], f32, tag="osb")
                nc.vector.tensor_tensor(osb[:], ps3[:], bl3_sb[:], ALU.add)
                nc.sync.dma_start(OUT[:], osb[:])

    nc.compile()
    return nc


def _get_module():
    if "nc" not in _CACHE:
        _CACHE["nc"] = _build_module()
    return _CACHE["nc"]


def _prep_weights(inp):
    """Host-side weight preprocessing -> dict of DRAM tensor arrays (fp32)."""
    f = np.float32
    out = {}
    ws = [
        (inp["w1"], inp["g1"], inp["b1"]),
        (inp["w2"], inp["g2"], inp["b2"]),
        (inp["w3"], inp["g3"], inp["b3"]),
        (inp["w4"], inp["g4"], inp["b4"]),
    ]
    for li, ((w, g, b), (ci, co)) in enumerate(zip(ws, DIMS)):
        w = np.asarray(w, f)
        nb = (co + 127) // 128
        cb = min(co, 128)
        d_, pp, _ = GATH[li]
        wnT = w[:, :ci].T  # [ci, co]
        cols = [np.tile(wnT[:, pp * dd : pp * (dd + 1)], (1, 128 // pp)) for dd in range(d_)]
        out[f"wn{li}"] = np.ascontiguousarray(np.concatenate(cols, axis=1))
        out[f"wd{li}"] = np.ascontiguousarray((w[:, ci:] - w[:, :ci]).T)
        out[f"gs{li}"] = np.ascontiguousarray(
            (np.asarray(g, f) * BN_SCALE).reshape(nb, cb).T
        )
        out[f"bb{li}"] = np.ascontiguousarray(np.asarray(b, f).reshape(nb, cb).T)
    w5 = np.asarray(inp["w5"], f)  # [1024, 512]
    w5t = w5.T  # [512, 1024]
    w5t_rs = np.zeros((128, 5, 1024), f)
    w5t_rs[0:64, 0, :] = w5t[0:64]
    w5t_rs[0:64, 1, :] = w5t[64:128]
    w5t_rs[:, 2, :] = w5t[128:256]
    w5t_rs[:, 3, :] = w5t[256:384]
    w5t_rs[:, 4, :] = w5t[384:512]
    out["w5t"] = w5t_rs
    out["g5s"] = np.ascontiguousarray(
        (np.asarray(inp["g5"], f) * BN_SCALE).reshape(8, 128).T
    )
    out["b5s"] = np.ascontiguousarray(np.asarray(inp["b5"], f).reshape(8, 128).T)
    wl1 = np.asarray(inp["wl1"], f).copy()  # [512, 2048]
    wl1[:, 1024:] *= f(1.0 / N)  # fold the mean-pool division
    out["wl1t"] = np.ascontiguousarray(wl1.T.reshape(16, 128, 512).transpose(1, 0, 2))
    out["g6s"] = np.ascontiguousarray(
        (np.asarray(inp["g6"], f) * BN_SCALE).reshape(4, 128).T
    )
    out["b6s"] = np.ascontiguousarray(np.asarray(inp["b6"], f).reshape(4, 128).T)
    wl2 = np.asarray(inp["wl2"], f)  # [256, 512]
    out["wl2t"] = np.ascontiguousarray(wl2.T.reshape(4, 128, 256).transpose(1, 0, 2))
    g7s = np.asarray(inp["g7"], f) * BN_SCALE
    out["g7s"] = np.ascontiguousarray(g7s.reshape(2, 128).T)
    bi2 = np.asarray(inp["bl2"], f) * g7s + np.asarray(inp["b7"], f)
    out["bi2"] = np.ascontiguousarray(bi2.reshape(2, 128).T)
    wl3 = np.asarray(inp["wl3"], f)  # [40, 256]
    out["wl3t"] = np.ascontiguousarray(wl3.T.reshape(2, 128, 40).transpose(1, 0, 2))
    out["bl3s"] = np.ascontiguousarray(np.asarray(inp["bl3"], f).reshape(40, 1))
    return out


def kernel(**inputs):
    from concourse.bass_utils import run_bass_kernel_spmd

    nc = _get_module()
    wmap = _prep_weights(inputs)
    x = np.asarray(inputs["x"], np.float32)  # [8, 3, 2048]
    in_maps = []
    for c in range(NCORES):
        m = dict(wmap)
        m["xq"] = np.ascontiguousarray(x[c])
        in_maps.append(m)
    res = run_bass_kernel_spmd(nc, in_maps, core_ids=list(range(NCORES)))
    out = np.stack([res.results[c]["outq"].reshape(40) for c in range(NCORES)])
    return out.astype(np.float32)


if __name__ == "__main__":
    nc = _get_module()
    print("module built OK")

